# revision 12
# baseline (speedup 1.0000x reference)
"""Bahdanau attention kernel for Trainium2 (8 NeuronCores, data-parallel over batch).

reference:
    cat    = concat([enc, tile(hidden)], -1)          # [B, S, H]
    energy = tanh(cat @ W_attn.T + b_attn)            # [B, S, H]
    scores = energy @ v_w                             # [B, S]
    attn   = softmax(scores, axis=1)
    ctx    = attn @ enc                               # [B, D]

Host-side algebra: split W_attn = [W1 | W2] along its input dim. The W2 (hidden)
half collapses to a per-batch bias hb = hidden @ W2.T + b_attn, so on-device
work is tanh(enc @ W1.T + hb) -> v-dot -> softmax -> weighted sum.
Each core handles B/8 = 2 batches; no collectives.

Per 2048-seq supertile (4 x 512-column psum banks), per batch:
  energy : psum [o_chunk=128, s=2048]; loop oc(8)/kc(4)/j(4) with j innermost so
           4 consecutive matmuls share one stationary W1T chunk; a post-Tile
           pass (dedup_ldweights) deletes the redundant LDWEIGHTS.
  tanh   : ScalarE, per-partition bias hb[o], per (oc, j) bank, into
           e_all [128, oc, 2048] bf16.
  scores : per j: 8 v-stationary matmuls (one per oc) accumulate [1, 512].
  exp    : ScalarE Exp (scores bounded |s| <= ||v||_1 ~ 25, softmax is
           shift-invariant, so no max pass); accum_out emits the partial Z.
  p^T    : 4 K=1 matmuls ([1,128] x ones -> [128,1]) per j.
  ctx    : p-stationary matmuls accumulate [1, 512] across the batch.
  finish : Z = reduce(zbuf); ctx * (1/Z); DMA out.
"""

import numpy as np
import ml_dtypes

import concourse.bacc as bacc
import concourse.bass as bass
import concourse.tile as tile
from concourse import mybir
from concourse.bass_utils import run_bass_kernel_spmd

B, S, D, H = 16, 8192, 512, 1024
NCORES = 8
BPC = B // NCORES          # batches per core
ST = 2048                  # seq supertile
NT = S // ST               # supertiles per batch (4)
NJ = ST // 512             # psum-bank columns per supertile (4)
KC = D // 128              # contraction chunks (4)
OC = H // 128              # o-chunks (8)
NU = ST // 128             # 128-blocks per supertile (16)

F32 = mybir.dt.float32
BF16 = mybir.dt.bfloat16
BF = ml_dtypes.bfloat16
AX = mybir.AxisListType.X
TANH = mybir.ActivationFunctionType.Tanh
EXP = mybir.ActivationFunctionType.Exp

_CACHE = {}


def dedup_ldweights(nc):
    """Remove InstLdweights that reload the exact weights already resident.

    Runs after TileContext exit (on the final scheduled order) and before
    nc.compile(). Only drops LDWs with empty sync_info; the matching
    InstMatmult(ldweights=False) then uses the weights loaded by the earlier
    identical LDW. Keyed on the physical AP + mode fields.
    """
    removed = 0
    for blk in nc.m.functions[0].blocks:
        keep = []
        last_key = None
        for i in blk.instructions:
            if type(i).__name__ == 'InstLdweights':
                key = (str(i.ins[0]), str(i.is_transpose),
                       str(i.tile_position), str(i.perf_mode))
                empty_sync = i.sync_info is None or (
                    not i.sync_info.on_wait and not i.sync_info.on_update)
                if key == last_key and empty_sync:
                    removed += 1
                    continue
                last_key = key
            keep.append(i)
        blk.instructions = keep
    return removed


def build_nc():
    nc = bacc.Bacc(None, target_bir_lowering=False)
    encT = nc.declare_dram_parameter("encT", [BPC, D, S], BF16, isOutput=False)
    encN = nc.declare_dram_parameter("encN", [BPC, S, D], BF16, isOutput=False)
    w1t = nc.declare_dram_parameter("w1t", [D, H], BF16, isOutput=False)
    hb = nc.declare_dram_parameter("hb", [BPC, H], F32, isOutput=False)
    vw = nc.declare_dram_parameter("vw", [H], BF16, isOutput=False)
    out = nc.declare_dram_parameter("out", [BPC, D], F32, isOutput=True)

    with tile.TileContext(nc) as tc:
        with (
            tc.tile_pool(name="singles", bufs=1) as singles,
            tc.tile_pool(name="enc", bufs=2) as encp,
            tc.tile_pool(name="energy", bufs=2) as enp,
            tc.tile_pool(name="small", bufs=4) as smallp,
            tc.tile_pool(name="batch", bufs=2) as batchp,
            tc.tile_pool(name="eps", bufs=1, space="PSUM") as eps_pool,
            tc.tile_pool(name="scps", bufs=2, space="PSUM") as scps_pool,
            tc.tile_pool(name="ptps", bufs=1, space="PSUM") as ptps_pool,
            tc.tile_pool(name="ctxps", bufs=1, space="PSUM") as ctxps_pool,
        ):
            # ---- setup (once) ----
            w1t_sb = singles.tile([128, KC, H], BF16)
            nc.sync.dma_start(
                out=w1t_sb, in_=w1t.rearrange("(kc p) o -> p kc o", p=128)
            )
            v_sb = singles.tile([128, OC], BF16)
            nc.gpsimd.dma_start(out=v_sb, in_=vw.rearrange("(oc p) -> p oc", p=128))
            hb_sb = singles.tile([128, BPC, OC], F32)
            nc.gpsimd.dma_start(
                out=hb_sb, in_=hb.rearrange("b (oc p) -> p b oc", p=128)
            )
            ones_sb = singles.tile([1, 1], BF16)
            nc.vector.memset(ones_sb, 1.0)

            for b in range(BPC):
                ctx_ps = ctxps_pool.tile([1, D], F32)
                zbuf = batchp.tile([1, NT * NJ], F32)
                for t in range(NT):
                    # ---- loads ----
                    encT_sb = encp.tile([128, KC, ST], BF16)
                    nc.sync.dma_start(
                        out=encT_sb,
                        in_=encT[b, :, t * ST:(t + 1) * ST].rearrange(
                            "(kc p) s -> p kc s", p=128
                        ),
                    )
                    encN_sb = encp.tile([128, NU, D], BF16)
                    nc.sync.dma_start(
                        out=encN_sb,
                        in_=encN[b, t * ST:(t + 1) * ST, :].rearrange(
                            "(u p) d -> p u d", p=128
                        ),
                    )
                    # ---- energy: 4 consecutive matmuls per W1T chunk ----
                    e_ps = eps_pool.tile([128, ST], F32)
                    e_all = enp.tile([128, OC, ST], BF16)
                    for oc in range(OC):
                        for kc in range(KC):
                            for j in range(NJ):
                                nc.tensor.matmul(
                                    e_ps[:, j * 512:(j + 1) * 512],
                                    lhsT=w1t_sb[:, kc, oc * 128:(oc + 1) * 128],
                                    rhs=encT_sb[:, kc, j * 512:(j + 1) * 512],
                                    start=(kc == 0),
                                    stop=(kc == KC - 1),
                                )
                        for j in range(NJ):
                            nc.scalar.activation(
                                e_all[:, oc, j * 512:(j + 1) * 512],
                                e_ps[:, j * 512:(j + 1) * 512],
                                TANH, bias=hb_sb[:, b, oc:oc + 1],
                            )
                    # ---- scores / softmax / context per 512-column ----
                    for j in range(NJ):
                        sc_ps = scps_pool.tile([1, 512], F32)
                        for oc in range(OC):
                            nc.tensor.matmul(
                                sc_ps,
                                lhsT=v_sb[:, oc:oc + 1],
                                rhs=e_all[:, oc, j * 512:(j + 1) * 512],
                                start=(oc == 0),
                                stop=(oc == OC - 1),
                            )
                        p_sb = smallp.tile([1, 512], BF16)
                        nc.scalar.activation(
                            p_sb, sc_ps, EXP,
                            accum_out=zbuf[:, t * NJ + j:t * NJ + j + 1],
                        )
                        pt_ps = ptps_pool.tile([128, NJ], F32)
                        for i in range(4):
                            nc.tensor.matmul(
                                pt_ps[:, i:i + 1],
                                lhsT=p_sb[:, i * 128:(i + 1) * 128],
                                rhs=ones_sb,
                                start=True, stop=True,
                            )
                        pt_sb = smallp.tile([128, NJ], BF16)
                        nc.vector.tensor_copy(pt_sb, pt_ps)
                        for i in range(4):
                            u = j * 4 + i
                            nc.tensor.matmul(
                                ctx_ps,
                                lhsT=pt_sb[:, i:i + 1],
                                rhs=encN_sb[:, u, :],
                                start=(t == 0 and u == 0),
                                stop=(t == NT - 1 and u == NU - 1),
                            )
                # ---- normalize + store ----
                zsum = smallp.tile([1, 1], F32)
                nc.vector.reduce_sum(out=zsum, in_=zbuf, axis=AX)
                rz = smallp.tile([1, 1], F32)
                nc.vector.reciprocal(rz, zsum)
                ctx_sb = smallp.tile([1, D], F32)
                nc.vector.tensor_scalar_mul(ctx_sb, ctx_ps, rz)
                nc.sync.dma_start(out=out[b:b + 1, :], in_=ctx_sb)
    dedup_ldweights(nc)
    nc.compile()
    return nc


def _prep_inputs(encoder_outputs, hidden, W_attn, b_attn, v_w):
    enc_bf = encoder_outputs.astype(BF)
    encT_bf = np.ascontiguousarray(enc_bf.transpose(0, 2, 1))
    hb = (hidden.astype(np.float64) @ W_attn[:, D:].T.astype(np.float64)
          + b_attn.astype(np.float64)).astype(np.float32)
    w1t_bf = np.ascontiguousarray(W_attn[:, :D].T).astype(BF)
    v_bf = v_w.astype(BF)
    in_maps = []
    for c in range(NCORES):
        sl = slice(c * BPC, (c + 1) * BPC)
        in_maps.append({
            "encT": encT_bf[sl],
            "encN": enc_bf[sl],
            "w1t": w1t_bf,
            "hb": hb[sl],
            "vw": v_bf,
        })
    return in_maps


def _run(in_maps, trace=False):
    if "nc" not in _CACHE:
        _CACHE["nc"] = build_nc()
    nc = _CACHE["nc"]
    kw = {}
    if trace:
        import os
        import shutil
        shutil.rmtree("/tmp/bass_trace", ignore_errors=True)
        os.makedirs("/tmp/bass_trace", exist_ok=True)
        kw = {"tmpdir": "/tmp/bass_trace"}
    res = run_bass_kernel_spmd(nc, in_maps, list(range(NCORES)), trace=trace, **kw)
    out = np.concatenate([res.results[c]["out"] for c in range(NCORES)], axis=0)
    return out.astype(np.float32), res


def kernel(**inputs):
    in_maps = _prep_inputs(
        inputs["encoder_outputs"], inputs["hidden"], inputs["W_attn"],
        inputs["b_attn"], inputs["v_w"],
    )
    out, _ = _run(in_maps, trace=False)
    return out


def kernel_traced(**inputs):
    """test.py entry: also returns BassKernelResults with profile info."""
    in_maps = _prep_inputs(
        inputs["encoder_outputs"], inputs["hidden"], inputs["W_attn"],
        inputs["b_attn"], inputs["v_w"],
    )
    return _run(in_maps, trace=True)
